# revision 2
# baseline (speedup 1.0000x reference)
"""FAVOR+ (Performer) causal linear attention with rotary embeddings on 8 TRN2 cores.

Reference computation (B=2, L=4096, H=8, D=64, M=256):
  q,k <- GPT-J rotary(q, k, sinu_pos)
  qp = relu(rot_q @ projT / sqrt(M)) + EPS   [B,L,H,M]
  kp = relu(rot_k @ projT / sqrt(M)) + EPS
  causal scan over L: KV_l = sum_{j<=l} kp_j (x) [v_j, 1];  out_l = (qp_l @ KV_l)[:D] / (qp_l @ KV_l)[D]

Sharding: 16 (b,h) pairs, 2 per core (pure data parallel, no collectives).

v5 design (from the v3 kernel at ~123-141us; trace showed PE 75% busy at
cold clock with Vector MAX,ADD 1.46us + Scalar RELU/COPYs ~2.4us per chunk
serializing against it):
 - Features computed ONCE in bf16 WITH +EPS baked in (single DVE
   tensor_scalar per side); every consumer (AT, po, KV) uses them, so the
   l-major kp relu (Scalar), the separate fp8 path, and the EPS rank-1
   state matmul all disappear.
 - l-major kp for the KV update is produced by DMA TRANSPOSE (SBUF->SBUF
   bf16 xbar) of the m-major k features: kills the 2 pfkp matmuls per
   chunk and runs on otherwise-idle DMA engines.
 - po is computed UNTRANSPOSED: po[lq, 0:68] = qp^T KV_snap + AT^T v_aug
   via matmuls whose moving operands are only 68 wide (kv_sb / v_aug),
   with the masked AT (bf16, pair-merged single DVE multiply) as the
   stationary of the in-chunk part. Output is l-major: one merged
   [128, 136] ACT copy -> resident obuf, 4 big quarter DMAs out.
 - PSUM: pfqk 2banks x2bufs + kv 1 + (atpo|po) 1bank x2bufs = 7 of 8.
Measured end-to-end rel err vs fp64 reference: ~1.5e-3 (all-bf16 beats
the v3 fp8 path's 6.1e-3).
"""

import sys
import os

for _p in ("/opt/trn_rl_repo", "/root/.axon_site/_ro/trn_rl_repo"):
    if os.path.isdir(_p) and _p not in sys.path:
        sys.path.insert(0, _p)

import numpy as np
import ml_dtypes
import concourse.bass as bass
import concourse.mybir as mybir
import concourse.tile as tile
from concourse.bass_utils import run_bass_kernel_spmd

B, L, H, D, M = 2, 4096, 8, 64, 256
EPS = 1e-3
C = 128                 # chunk length
NCH = L // C            # 32 chunks
NCORES = 8
PAIRS_PER_CORE = (B * H) // NCORES  # 2
F32 = mybir.dt.float32
BF16 = mybir.dt.bfloat16
VW = 68                 # v_aug row width: 64 v + 1 ones + 3 zero pad


def _legalize_sync_waits(nc):
    """Split multi-wait instructions into preceding single-wait
    EventSemaphore ops on the same engine (same-engine execution is
    in-order, so sequential waits == AND of waits)."""
    for f in nc.m.functions:
        for b in f.blocks:
            insts = b.instructions
            new = []
            dirty = False
            for ins in insts:
                si = ins.sync_info
                if si is not None and si.on_wait is not None and len(si.on_wait) > 1:
                    waits = list(si.on_wait)
                    for j, wt in enumerate(waits[:-1]):
                        es = mybir.InstEventSemaphore(
                            name=f"{ins.name}_xw{j}",
                            engine=ins.engine,
                            ins=[],
                            outs=[],
                            sync_info=mybir.SyncInfo(on_wait=[wt], on_update=[]),
                        )
                        new.append(es)
                    ins.sync_info = mybir.SyncInfo(
                        on_wait=[waits[-1]], on_update=list(si.on_update or [])
                    )
                    dirty = True
                if si is not None and si.on_update is not None and len(si.on_update) > 1:
                    raise AssertionError(
                        f"multi-update on {ins.name} ({ins.opcode}) unsupported"
                    )
                new.append(ins)
            if dirty:
                b.instructions = new


def _build_program(legalize=True):
    nc = bass.Bass()

    xtb_d = nc.dram_tensor("xtb", [128, PAIRS_PER_CORE * L], BF16, kind="ExternalInput")
    vp_d = []
    for p in range(PAIRS_PER_CORE):
        vp_d.append(nc.dram_tensor(f"vp{p}", [128, NCH * VW], BF16, kind="ExternalInput"))
    out_d = nc.dram_tensor(
        "o", [128, NCH * PAIRS_PER_CORE * VW], BF16, kind="ExternalOutput"
    )
    projs_d = nc.dram_tensor("projs", [128, M], BF16, kind="ExternalInput")
    mask_d = nc.dram_tensor("mask2", [C, 2 * C], BF16, kind="ExternalInput")

    with tile.TileContext(nc) as tc:
        with (
            tc.tile_pool(name="consts", bufs=1) as consts,
            tc.tile_pool(name="feat", bufs=2) as feat,
            tc.tile_pool(name="kplp", bufs=2) as kplp,
            tc.tile_pool(name="state", bufs=2) as state,
            tc.tile_pool(name="persist", bufs=1) as persist,
            tc.tile_pool(name="psQK", bufs=2, space="PSUM") as psQK,
            tc.tile_pool(name="psKV", bufs=1, space="PSUM") as psKV,
            tc.tile_pool(name="psAP", bufs=2, space="PSUM") as psAP,
        ):
            # ---- resident inputs / constants ----
            # small constants FIRST: the very first feature matmul needs
            # projs, and the HWDGE ring drains in emission order
            projs = consts.tile([128, M], BF16)
            nc.sync.dma_start(projs[:], projs_d[:])
            mask2 = consts.tile([C, 2 * C], BF16)
            nc.sync.dma_start(mask2[:], mask_d[:])
            # xtb: [128, pair, L]; rows 0:64 = rot_q^T, rows 64:128 = rot_k^T
            # sub-DMAs ordered so BOTH pairs' early chunks land first
            # (pair 1's chunk 0 lives at flat col 4096)
            xtb = consts.tile([128, PAIRS_PER_CORE, L], BF16, name="xtb", tag="xtb")
            for g in (0, 4):
                nc.sync.dma_start(
                    xtb[:].rearrange("p a l -> p (a l)")[:, g * 1024 : (g + 1) * 1024],
                    xtb_d[:, g * 1024 : (g + 1) * 1024],
                )
            half = (NCH // 2) * VW
            vp_all = consts.tile([128, PAIRS_PER_CORE, NCH, VW], BF16,
                                 name="vpall", tag="vpall")
            for p in range(PAIRS_PER_CORE):
                nc.sync.dma_start(
                    vp_all[:, p, 0 : NCH // 2, :],
                    vp_d[p][:, 0:half].rearrange("p (c w) -> p c w", w=VW),
                )
            for g in (1, 5, 2, 6, 3, 7):
                nc.sync.dma_start(
                    xtb[:].rearrange("p a l -> p (a l)")[:, g * 1024 : (g + 1) * 1024],
                    xtb_d[:, g * 1024 : (g + 1) * 1024],
                )
            for p in range(PAIRS_PER_CORE):
                nc.sync.dma_start(
                    vp_all[:, p, NCH // 2 : NCH, :],
                    vp_d[p][:, half : 2 * half].rearrange("p (c w) -> p c w", w=VW),
                )
            # resident output accumulation buffer [l, chunk, (pair, 68)]
            obuf = consts.tile([128, NCH, PAIRS_PER_CORE * VW], BF16,
                               name="obuf", tag="obuf")

            # KV state, both pairs in one bank: pair p half h at col (2p+h)*68
            kv_ps = psKV.tile([128, 4 * VW], F32, name="kvps", tag="kvps")
            kv_sb = persist.tile([128, 4 * VW], BF16, name="kvsb", tag="kvsb")

            def stage_a(ci):
                """bf16 features (relu + EPS) for chunk ci, both pairs,
                k side first; l-major kp via DMA transpose."""
                lo = ci * C
                pfqk = psQK.tile([128, 1024], F32, tag="pfqk", name=f"pfqk{ci}")
                # cols: side*512 + h*256 + p*128 + l   (side 1 = k first)
                for side in (1, 0):
                    for h in range(2):
                        nc.tensor.matmul(
                            pfqk[:, side * 512 + h * 256 : side * 512 + (h + 1) * 256],
                            projs[side * D : (side + 1) * D, h * 128 : (h + 1) * 128],
                            xtb[side * D : (side + 1) * D, :, lo : lo + C],
                            start=True, stop=True,
                        )
                fs = feat.tile([128, 2, 512], BF16, tag="fs", name=f"fs{ci}")
                nc.vector.tensor_scalar(
                    fs[:, 1, :], pfqk[:, 512:1024],
                    0.0, EPS, mybir.AluOpType.max, mybir.AluOpType.add,
                )
                nc.vector.tensor_scalar(
                    fs[:, 0, :], pfqk[:, 0:512],
                    0.0, EPS, mybir.AluOpType.max, mybir.AluOpType.add,
                )
                if ci < NCH - 1:
                    kpl = kplp.tile([128, 2, 2, C], BF16, tag="kpl", name=f"kpl{ci}")
                    for p in range(PAIRS_PER_CORE):
                        for h in range(2):
                            eng = nc.sync if p == 0 else nc.scalar
                            eng.dma_start(
                                kpl[:, p, h, :],
                                fs[:, 1, h * 256 + p * 128 : h * 256 + (p + 1) * 128],
                                transpose=True,
                            )
                else:
                    kpl = None
                return fs, kpl

            def stage_b(ci, fs, kpl):
                """Scan state + l-major output for chunk ci, both pairs."""
                t = psAP.tile([128, 392], F32, tag="atpo", name=f"ap{ci}")
                # in-chunk quadratic AT[lk, lq], pair p at cols p*128
                for p in range(PAIRS_PER_CORE):
                    for h in range(2):
                        nc.tensor.matmul(
                            t[:, p * 128 : (p + 1) * 128],
                            fs[:, 1, h * 256 + p * 128 : h * 256 + (p + 1) * 128],
                            fs[:, 0, h * 256 + p * 128 : h * 256 + (p + 1) * 128],
                            start=(h == 0), stop=(h == 1),
                        )
                at_sb = state.tile([C, 2, C], BF16, tag="atsb", name=f"at{ci}")
                nc.vector.tensor_tensor(
                    at_sb[:].rearrange("p a l -> p (a l)"),
                    t[:, 0:256], mask2[:], mybir.AluOpType.mult,
                )
                for p in range(PAIRS_PER_CORE):
                    po = t[:, 256 + p * VW : 256 + (p + 1) * VW]
                    vslice = vp_all[:, p, ci, :]
                    if ci > 0:
                        for h in range(2):
                            nc.tensor.matmul(
                                po,
                                fs[:, 0, h * 256 + p * 128 : h * 256 + (p + 1) * 128],
                                kv_sb[:, (2 * p + h) * VW : (2 * p + h + 1) * VW],
                                start=(h == 0), stop=False,
                            )
                        nc.tensor.matmul(
                            po, at_sb[:, p, :], vslice, start=False, stop=True
                        )
                    else:
                        nc.tensor.matmul(
                            po, at_sb[:, p, :], vslice, start=True, stop=True
                        )
                # KV += kp^T v_aug  (kp is l-major WITH eps via dma transpose)
                if ci < NCH - 1:
                    for p in range(PAIRS_PER_CORE):
                        vslice = vp_all[:, p, ci, :]
                        for h in range(2):
                            nc.tensor.matmul(
                                kv_ps[:, (2 * p + h) * VW : (2 * p + h + 1) * VW],
                                kpl[:, p, h, :], vslice,
                                start=(ci == 0 and p == 0 and h == 0), stop=True,
                                skip_group_check=True,
                            )
                nc.scalar.activation(
                    obuf[:, ci, :], t[:, 256:392],
                    mybir.ActivationFunctionType.Copy,
                )

            # software pipeline: A(ci) one chunk ahead of B(ci-1).
            # The KV snapshot for B(ci-1) is emitted BEFORE A(ci): its data
            # (kv updates of ci-2) is long ready, hoisting it removes ACT
            # queue delay from the scan spine (v3 lesson; ACT-only hoist).
            pend = {}
            for ci in range(NCH):
                if ci >= 1:
                    nc.scalar.activation(
                        kv_sb[:], kv_ps[:], mybir.ActivationFunctionType.Copy
                    )
                pend[ci] = stage_a(ci)
                if ci >= 1:
                    stage_b(ci - 1, *pend.pop(ci - 1))
                # drain finished output quarters early
                if ci in (NCH // 4 + 1, NCH // 2 + 1, 3 * NCH // 4 + 1):
                    qs = (ci - 1) - NCH // 4
                    nc.scalar.dma_start(
                        out_d[:, qs * 2 * VW : (qs + NCH // 4) * 2 * VW],
                        obuf[:, qs : qs + NCH // 4, :],
                    )
                elif ci == NCH - 1:
                    nc.scalar.dma_start(
                        out_d[:, (3 * NCH // 4) * 2 * VW : (NCH - 1) * 2 * VW],
                        obuf[:, 3 * NCH // 4 : NCH - 1, :],
                    )
            stage_b(NCH - 1, *pend.pop(NCH - 1))
            nc.scalar.dma_start(
                out_d[:, (NCH - 1) * 2 * VW :],
                obuf[:, NCH - 1 :, :],
            )

    if legalize:
        _legalize_sync_waits(nc)
    return nc


_PROGRAM_CACHE = {}


def _get_program():
    if "nc" not in _PROGRAM_CACHE:
        _PROGRAM_CACHE["nc"] = _build_program()
    return _PROGRAM_CACHE["nc"]


def _host_rotary(q, k, sinu_pos):
    """Apply GPT-J rotary on host in fp32, return rot_q, rot_k [B,L,H,D]."""
    sinu = np.asarray(sinu_pos, np.float32)[0]          # [L, D]
    half = D // 2
    sin_i = np.repeat(sinu[:, :half], 2, axis=-1)       # [L, D]
    cos_i = np.repeat(sinu[:, half:], 2, axis=-1)

    def rot(t):
        t = np.asarray(t, np.float32)
        r = np.empty_like(t)
        r[..., 0::2] = -t[..., 1::2]
        r[..., 1::2] = t[..., 0::2]
        c = cos_i[None, :, None, :]
        s = sin_i[None, :, None, :]
        return t * c + r * s

    return rot(q), rot(k)


def build_in_maps(q, k, v, sinu_pos, proj):
    bf = ml_dtypes.bfloat16
    rq, rk = _host_rotary(q, k, sinu_pos)
    v = np.asarray(v, np.float32)
    proj = np.asarray(proj, np.float32)

    ratio = 1.0 / np.sqrt(np.float32(M))
    projs = np.zeros((128, M), np.float32)
    projs[0:D, :] = ratio * proj.T
    projs[D : 2 * D, :] = ratio * proj.T
    mask2 = np.tile(np.triu(np.ones((C, C), np.float32)), (1, 2))

    pairs = [(b, h) for b in range(B) for h in range(H)]
    in_maps = []
    for core in range(NCORES):
        im = {
            "projs": projs.astype(bf),
            "mask2": mask2.astype(bf),
        }
        xtb = np.empty((128, PAIRS_PER_CORE, L), np.float32)
        for p in range(PAIRS_PER_CORE):
            b, h = pairs[core * PAIRS_PER_CORE + p]
            xtb[0:D, p, :] = rq[b, :, h, :].T
            xtb[D : 2 * D, p, :] = rk[b, :, h, :].T
            vz = np.zeros((C, NCH, VW), np.float32)
            vz[:, :, 0:D] = v[b, :, h, :].reshape(NCH, C, D).transpose(1, 0, 2)
            vz[:, :, D] = 1.0
            im[f"vp{p}"] = np.ascontiguousarray(
                vz.reshape(C, NCH * VW)
            ).astype(bf)
        im["xtb"] = np.ascontiguousarray(
            xtb.reshape(128, PAIRS_PER_CORE * L)
        ).astype(bf)
        in_maps.append(im)
    return in_maps


def kernel(q, k, v, sinu_pos, proj):
    nc = _get_program()
    in_maps = build_in_maps(q, k, v, sinu_pos, proj)
    res = run_bass_kernel_spmd(nc, in_maps, core_ids=list(range(NCORES)))

    pairs = [(b, h) for b in range(B) for h in range(H)]
    out = np.empty((B, L, H, D), np.float32)
    for core in range(NCORES):
        ob = np.asarray(res.results[core]["o"], dtype=np.float32).reshape(
            128, NCH, PAIRS_PER_CORE, VW
        )
        for p in range(PAIRS_PER_CORE):
            b, h = pairs[core * PAIRS_PER_CORE + p]
            x = ob[:, :, p, :].transpose(1, 0, 2).reshape(L, VW)  # [L, 68]
            out[b, :, h, :] = x[:, 0:D] / x[:, D : D + 1]
    return out


# revision 8
# speedup vs baseline: 1.8582x; 1.8582x over previous
"""FAVOR+ (Performer) causal linear attention with rotary embeddings on 8 TRN2 cores.

Reference computation (B=2, L=4096, H=8, D=64, M=256):
  q,k <- GPT-J rotary(q, k, sinu_pos)
  qp = relu(rot_q @ projT / sqrt(M)) + EPS   [B,L,H,M]
  kp = relu(rot_k @ projT / sqrt(M)) + EPS
  causal scan over L: KV_l = sum_{j<=l} kp_j (x) [v_j, 1];  out_l = (qp_l @ KV_l)[:D] / (qp_l @ KV_l)[D]

Sharding: 16 (b,h) pairs, 2 per core (pure data parallel, no collectives).

v5 design (from the v3 kernel at ~123-141us; trace showed PE 75% busy at
cold clock with Vector MAX,ADD 1.46us + Scalar RELU/COPYs ~2.4us per chunk
serializing against it):
 - Features computed ONCE in bf16 WITH +EPS baked in (single DVE
   tensor_scalar per side); every consumer (AT, po, KV) uses them, so the
   l-major kp relu (Scalar), the separate fp8 path, and the EPS rank-1
   state matmul all disappear.
 - l-major kp for the KV update is produced by DMA TRANSPOSE (SBUF->SBUF
   bf16 xbar) of the m-major k features: kills the 2 pfkp matmuls per
   chunk and runs on otherwise-idle DMA engines.
 - po is computed UNTRANSPOSED: po[lq, 0:68] = qp^T KV_snap + AT^T v_aug
   via matmuls whose moving operands are only 68 wide (kv_sb / v_aug),
   with the masked AT (bf16, pair-merged single DVE multiply) as the
   stationary of the in-chunk part. Output is l-major: one merged
   [128, 136] ACT copy -> resident obuf, 4 big quarter DMAs out.
 - PSUM: pfqk 2banks x2bufs + kv 1 + (atpo|po) 1bank x2bufs = 7 of 8.
Measured end-to-end rel err vs fp64 reference: ~1.5e-3 (all-bf16 beats
the v3 fp8 path's 6.1e-3).
"""

import sys
import os

for _p in ("/opt/trn_rl_repo", "/root/.axon_site/_ro/trn_rl_repo"):
    if os.path.isdir(_p) and _p not in sys.path:
        sys.path.insert(0, _p)

import numpy as np
import ml_dtypes
import concourse.bass as bass
import concourse.mybir as mybir
import concourse.tile as tile
from concourse.bass_utils import run_bass_kernel_spmd

B, L, H, D, M = 2, 4096, 8, 64, 256
EPS = 1e-3
C = 128                 # chunk length
NCH = L // C            # 32 chunks
NCORES = 8
PAIRS_PER_CORE = (B * H) // NCORES  # 2
F32 = mybir.dt.float32
BF16 = mybir.dt.bfloat16
VW = 68                 # v_aug row width: 64 v + 1 ones + 3 zero pad


def _legalize_sync_waits(nc):
    """Split multi-wait instructions into preceding single-wait
    EventSemaphore ops on the same engine (same-engine execution is
    in-order, so sequential waits == AND of waits)."""
    for f in nc.m.functions:
        for b in f.blocks:
            insts = b.instructions
            new = []
            dirty = False
            for ins in insts:
                si = ins.sync_info
                if si is not None and si.on_wait is not None and len(si.on_wait) > 1:
                    waits = list(si.on_wait)
                    for j, wt in enumerate(waits[:-1]):
                        es = mybir.InstEventSemaphore(
                            name=f"{ins.name}_xw{j}",
                            engine=ins.engine,
                            ins=[],
                            outs=[],
                            sync_info=mybir.SyncInfo(on_wait=[wt], on_update=[]),
                        )
                        new.append(es)
                    ins.sync_info = mybir.SyncInfo(
                        on_wait=[waits[-1]], on_update=list(si.on_update or [])
                    )
                    dirty = True
                if si is not None and si.on_update is not None and len(si.on_update) > 1:
                    raise AssertionError(
                        f"multi-update on {ins.name} ({ins.opcode}) unsupported"
                    )
                new.append(ins)
            if dirty:
                b.instructions = new


def _build_program(legalize=True):
    nc = bass.Bass()

    xtb_d = nc.dram_tensor("xtb", [128, PAIRS_PER_CORE * L], BF16, kind="ExternalInput")
    vp_d = []
    for p in range(PAIRS_PER_CORE):
        vp_d.append(nc.dram_tensor(f"vp{p}", [128, NCH * VW], BF16, kind="ExternalInput"))
    out_d = nc.dram_tensor(
        "o", [128, NCH * PAIRS_PER_CORE * VW], BF16, kind="ExternalOutput"
    )
    projs_d = nc.dram_tensor("projs", [128, M], BF16, kind="ExternalInput")
    mask_d = nc.dram_tensor("mask2", [C, 2 * C], BF16, kind="ExternalInput")
    epso_d = nc.dram_tensor("epsones", [128, 128], BF16, kind="ExternalInput")

    with tile.TileContext(nc) as tc:
        with (
            tc.tile_pool(name="consts", bufs=1) as consts,
            tc.tile_pool(name="feat", bufs=2) as feat,
            tc.tile_pool(name="kplp", bufs=2) as kplp,
            tc.tile_pool(name="state", bufs=2) as state,
            tc.tile_pool(name="persist", bufs=1) as persist,
            tc.tile_pool(name="psQK", bufs=2, space="PSUM") as psQK,
            tc.tile_pool(name="psKP", bufs=1, space="PSUM") as psKP,
            tc.tile_pool(name="psKV", bufs=1, space="PSUM") as psKV,
            tc.tile_pool(name="psAP", bufs=2, space="PSUM") as psAP,
        ):
            # ---- resident inputs / constants ----
            # small constants FIRST: the very first feature matmul needs
            # projs, and the HWDGE ring drains in emission order
            projs = consts.tile([128, M], BF16)
            nc.sync.dma_start(projs[:], projs_d[:])
            mask2 = consts.tile([C, 2 * C], BF16)
            nc.sync.dma_start(mask2[:], mask_d[:])
            epso = consts.tile([128, 128], BF16)
            nc.sync.dma_start(epso[:], epso_d[:])
            # xtb: [128, pair, L]; rows 0:64 = rot_q^T, rows 64:128 = rot_k^T
            # sub-DMAs ordered so BOTH pairs' early chunks land first
            # (pair 1's chunk 0 lives at flat col 4096)
            xtb = consts.tile([128, PAIRS_PER_CORE, L], BF16, name="xtb", tag="xtb")
            for g in (0, 4):
                nc.sync.dma_start(
                    xtb[:].rearrange("p a l -> p (a l)")[:, g * 1024 : (g + 1) * 1024],
                    xtb_d[:, g * 1024 : (g + 1) * 1024],
                )
            half = (NCH // 2) * VW
            vp_all = consts.tile([128, PAIRS_PER_CORE, NCH, VW], BF16,
                                 name="vpall", tag="vpall")
            for p in range(PAIRS_PER_CORE):
                nc.sync.dma_start(
                    vp_all[:, p, 0 : NCH // 2, :],
                    vp_d[p][:, 0:half].rearrange("p (c w) -> p c w", w=VW),
                )
            for g in (1, 5, 2, 6, 3, 7):
                nc.sync.dma_start(
                    xtb[:].rearrange("p a l -> p (a l)")[:, g * 1024 : (g + 1) * 1024],
                    xtb_d[:, g * 1024 : (g + 1) * 1024],
                )
            for p in range(PAIRS_PER_CORE):
                nc.sync.dma_start(
                    vp_all[:, p, NCH // 2 : NCH, :],
                    vp_d[p][:, half : 2 * half].rearrange("p (c w) -> p c w", w=VW),
                )
            # resident output accumulation buffer [l, chunk, (pair, 68)]
            obuf = consts.tile([128, NCH, PAIRS_PER_CORE * VW], BF16,
                               name="obuf", tag="obuf")

            # KV state, both pairs in one bank: pair p half h at col (2p+h)*68
            kv_ps = psKV.tile([128, 4 * VW], F32, name="kvps", tag="kvps")
            kv_sb = persist.tile([128, 4 * VW], BF16, name="kvsb", tag="kvsb")

            def stage_a(ci):
                """bf16 features (relu + EPS) for chunk ci, both pairs,
                k side first; plus l-major kp (relu only, EPS enters the
                KV state via the epso rank-1 matmul in stage_b)."""
                lo = ci * C
                pfqk = psQK.tile([128, 1024], F32, tag="pfqk", name=f"pfqk{ci}")
                pfkp = psKP.tile([128, 512], F32, tag="pfkp", name=f"pfkp{ci}")
                # cols: side*512 + h*256 + p*128 + l   (side 1 = k first)
                for side in (1, 0):
                    for h in range(2):
                        nc.tensor.matmul(
                            pfqk[:, side * 512 + h * 256 : side * 512 + (h + 1) * 256],
                            projs[side * D : (side + 1) * D, h * 128 : (h + 1) * 128],
                            xtb[side * D : (side + 1) * D, :, lo : lo + C],
                            start=True, stop=True,
                        )
                if ci < NCH - 1:
                    for p in range(PAIRS_PER_CORE):
                        nc.tensor.matmul(
                            pfkp[:, p * 256 : (p + 1) * 256],
                            xtb[D : 2 * D, p, lo : lo + C],
                            projs[D : 2 * D, :],
                            start=True, stop=True,
                        )
                fs = feat.tile([128, 2, 512], BF16, tag="fs", name=f"fs{ci}")
                nc.vector.tensor_scalar(
                    fs[:, 1, :], pfqk[:, 512:1024],
                    0.0, EPS, mybir.AluOpType.max, mybir.AluOpType.add,
                )
                nc.vector.tensor_scalar(
                    fs[:, 0, :], pfqk[:, 0:512],
                    0.0, EPS, mybir.AluOpType.max, mybir.AluOpType.add,
                )
                if ci < NCH - 1:
                    kpl = kplp.tile([128, 2, 2, C], BF16, tag="kpl", name=f"kpl{ci}")
                    nc.scalar.activation(
                        kpl[:], pfkp[:].rearrange("p (a b m) -> p a b m", a=2, b=2),
                        mybir.ActivationFunctionType.Relu,
                    )
                else:
                    kpl = None
                return fs, kpl

            def stage_b(ci, fs, kpl):
                """Scan state + l-major output for chunk ci, both pairs."""
                t = psAP.tile([128, 392], F32, tag="atpo", name=f"ap{ci}")
                # in-chunk quadratic AT[lk, lq], pair p at cols p*128
                for p in range(PAIRS_PER_CORE):
                    for h in range(2):
                        nc.tensor.matmul(
                            t[:, p * 128 : (p + 1) * 128],
                            fs[:, 1, h * 256 + p * 128 : h * 256 + (p + 1) * 128],
                            fs[:, 0, h * 256 + p * 128 : h * 256 + (p + 1) * 128],
                            start=(h == 0), stop=(h == 1),
                        )
                at_sb = state.tile([C, 2, C], BF16, tag="atsb", name=f"at{ci}")
                nc.vector.tensor_tensor(
                    at_sb[:].rearrange("p a l -> p (a l)"),
                    t[:, 0:256], mask2[:], mybir.AluOpType.mult,
                )
                for p in range(PAIRS_PER_CORE):
                    po = t[:, 256 + p * VW : 256 + (p + 1) * VW]
                    vslice = vp_all[:, p, ci, :]
                    if ci > 0:
                        for h in range(2):
                            nc.tensor.matmul(
                                po,
                                fs[:, 0, h * 256 + p * 128 : h * 256 + (p + 1) * 128],
                                kv_sb[:, (2 * p + h) * VW : (2 * p + h + 1) * VW],
                                start=(h == 0), stop=False,
                            )
                        nc.tensor.matmul(
                            po, at_sb[:, p, :], vslice, start=False, stop=True
                        )
                    else:
                        nc.tensor.matmul(
                            po, at_sb[:, p, :], vslice, start=True, stop=True
                        )
                # KV += kp^T v_aug + EPS * colsum(v_aug)
                if ci < NCH - 1:
                    for p in range(PAIRS_PER_CORE):
                        vslice = vp_all[:, p, ci, :]
                        for h in range(2):
                            nc.tensor.matmul(
                                kv_ps[:, (2 * p + h) * VW : (2 * p + h + 1) * VW],
                                kpl[:, p, h, :], vslice,
                                start=(ci == 0 and p == 0 and h == 0), stop=True,
                                skip_group_check=True,
                            )
                    # one EPS mm for BOTH pairs: flat [128, 272] out, the
                    # v slice broadcast over the half-dup dim per pair
                    nc.tensor.matmul(
                        kv_ps[:].rearrange("p (a b w) -> p a b w", a=2, b=2),
                        epso[:],
                        vp_all[:, :, ci : ci + 1, :]
                            .broadcast_to([128, PAIRS_PER_CORE, 2, VW]),
                        start=False, stop=True,
                        skip_group_check=True,
                    )
                nc.scalar.activation(
                    obuf[:, ci, :], t[:, 256:392],
                    mybir.ActivationFunctionType.Copy,
                )

            # software pipeline: A(ci) one chunk ahead of B(ci-1).
            # The KV snapshot for B(ci-1) is emitted BEFORE A(ci): its data
            # (kv updates of ci-2) is long ready, hoisting it removes ACT
            # queue delay from the scan spine (v3 lesson; ACT-only hoist).
            pend = {}
            for ci in range(NCH):
                if ci >= 1:
                    nc.scalar.activation(
                        kv_sb[:], kv_ps[:], mybir.ActivationFunctionType.Copy
                    )
                pend[ci] = stage_a(ci)
                if ci >= 1:
                    stage_b(ci - 1, *pend.pop(ci - 1))
                # drain finished output quarters early
                if ci in (NCH // 4 + 1, NCH // 2 + 1, 3 * NCH // 4 + 1):
                    qs = (ci - 1) - NCH // 4
                    nc.scalar.dma_start(
                        out_d[:, qs * 2 * VW : (qs + NCH // 4) * 2 * VW],
                        obuf[:, qs : qs + NCH // 4, :],
                    )
                elif ci == NCH - 1:
                    nc.scalar.dma_start(
                        out_d[:, (3 * NCH // 4) * 2 * VW : (NCH - 1) * 2 * VW],
                        obuf[:, 3 * NCH // 4 : NCH - 1, :],
                    )
            stage_b(NCH - 1, *pend.pop(NCH - 1))
            nc.scalar.dma_start(
                out_d[:, (NCH - 1) * 2 * VW :],
                obuf[:, NCH - 1 :, :],
            )

    if legalize:
        _legalize_sync_waits(nc)
    return nc


_PROGRAM_CACHE = {}


def _get_program():
    if "nc" not in _PROGRAM_CACHE:
        _PROGRAM_CACHE["nc"] = _build_program()
    return _PROGRAM_CACHE["nc"]


def _host_rotary(q, k, sinu_pos):
    """Apply GPT-J rotary on host in fp32, return rot_q, rot_k [B,L,H,D]."""
    sinu = np.asarray(sinu_pos, np.float32)[0]          # [L, D]
    half = D // 2
    sin_i = np.repeat(sinu[:, :half], 2, axis=-1)       # [L, D]
    cos_i = np.repeat(sinu[:, half:], 2, axis=-1)

    def rot(t):
        t = np.asarray(t, np.float32)
        r = np.empty_like(t)
        r[..., 0::2] = -t[..., 1::2]
        r[..., 1::2] = t[..., 0::2]
        c = cos_i[None, :, None, :]
        s = sin_i[None, :, None, :]
        return t * c + r * s

    return rot(q), rot(k)


def build_in_maps(q, k, v, sinu_pos, proj):
    bf = ml_dtypes.bfloat16
    rq, rk = _host_rotary(q, k, sinu_pos)
    v = np.asarray(v, np.float32)
    proj = np.asarray(proj, np.float32)

    ratio = 1.0 / np.sqrt(np.float32(M))
    projs = np.zeros((128, M), np.float32)
    projs[0:D, :] = ratio * proj.T
    projs[D : 2 * D, :] = ratio * proj.T
    mask2 = np.tile(np.triu(np.ones((C, C), np.float32)), (1, 2))
    epsones = np.full((128, 128), EPS, np.float32)

    pairs = [(b, h) for b in range(B) for h in range(H)]
    in_maps = []
    for core in range(NCORES):
        im = {
            "projs": projs.astype(bf),
            "mask2": mask2.astype(bf),
            "epsones": epsones.astype(bf),
        }
        xtb = np.empty((128, PAIRS_PER_CORE, L), np.float32)
        for p in range(PAIRS_PER_CORE):
            b, h = pairs[core * PAIRS_PER_CORE + p]
            xtb[0:D, p, :] = rq[b, :, h, :].T
            xtb[D : 2 * D, p, :] = rk[b, :, h, :].T
            vz = np.zeros((C, NCH, VW), np.float32)
            vz[:, :, 0:D] = v[b, :, h, :].reshape(NCH, C, D).transpose(1, 0, 2)
            vz[:, :, D] = 1.0
            im[f"vp{p}"] = np.ascontiguousarray(
                vz.reshape(C, NCH * VW)
            ).astype(bf)
        im["xtb"] = np.ascontiguousarray(
            xtb.reshape(128, PAIRS_PER_CORE * L)
        ).astype(bf)
        in_maps.append(im)
    return in_maps


def kernel(q, k, v, sinu_pos, proj):
    nc = _get_program()
    in_maps = build_in_maps(q, k, v, sinu_pos, proj)
    res = run_bass_kernel_spmd(nc, in_maps, core_ids=list(range(NCORES)))

    pairs = [(b, h) for b in range(B) for h in range(H)]
    out = np.empty((B, L, H, D), np.float32)
    for core in range(NCORES):
        ob = np.asarray(res.results[core]["o"], dtype=np.float32).reshape(
            128, NCH, PAIRS_PER_CORE, VW
        )
        for p in range(PAIRS_PER_CORE):
            b, h = pairs[core * PAIRS_PER_CORE + p]
            x = ob[:, :, p, :].transpose(1, 0, 2).reshape(L, VW)  # [L, 68]
            out[b, :, h, :] = x[:, 0:D] / x[:, D : D + 1]
    return out


# revision 10
# speedup vs baseline: 1.8796x; 1.0116x over previous
"""FAVOR+ (Performer) causal linear attention with rotary embeddings on 8 TRN2 cores.

Reference computation (B=2, L=4096, H=8, D=64, M=256):
  q,k <- GPT-J rotary(q, k, sinu_pos)
  qp = relu(rot_q @ projT / sqrt(M)) + EPS   [B,L,H,M]
  kp = relu(rot_k @ projT / sqrt(M)) + EPS
  causal scan over L: KV_l = sum_{j<=l} kp_j (x) [v_j, 1];  out_l = (qp_l @ KV_l)[:D] / (qp_l @ KV_l)[D]

Sharding: 16 (b,h) pairs, 2 per core (pure data parallel, no collectives).

v5 design (from the v3 kernel at ~123-141us; trace showed PE 75% busy at
cold clock with Vector MAX,ADD 1.46us + Scalar RELU/COPYs ~2.4us per chunk
serializing against it):
 - Features computed ONCE in bf16 WITH +EPS baked in (single DVE
   tensor_scalar per side); every consumer (AT, po, KV) uses them, so the
   l-major kp relu (Scalar), the separate fp8 path, and the EPS rank-1
   state matmul all disappear.
 - l-major kp for the KV update is produced by DMA TRANSPOSE (SBUF->SBUF
   bf16 xbar) of the m-major k features: kills the 2 pfkp matmuls per
   chunk and runs on otherwise-idle DMA engines.
 - po is computed UNTRANSPOSED: po[lq, 0:68] = qp^T KV_snap + AT^T v_aug
   via matmuls whose moving operands are only 68 wide (kv_sb / v_aug),
   with the masked AT (bf16, pair-merged single DVE multiply) as the
   stationary of the in-chunk part. Output is l-major: one merged
   [128, 136] ACT copy -> resident obuf, 4 big quarter DMAs out.
 - PSUM: pfqk 2banks x2bufs + kv 1 + (atpo|po) 1bank x2bufs = 7 of 8.
Measured end-to-end rel err vs fp64 reference: ~1.5e-3 (all-bf16 beats
the v3 fp8 path's 6.1e-3).
"""

import sys
import os

for _p in ("/opt/trn_rl_repo", "/root/.axon_site/_ro/trn_rl_repo"):
    if os.path.isdir(_p) and _p not in sys.path:
        sys.path.insert(0, _p)

import numpy as np
import ml_dtypes
import concourse.bass as bass
import concourse.mybir as mybir
import concourse.tile as tile
from concourse.bass_utils import run_bass_kernel_spmd

B, L, H, D, M = 2, 4096, 8, 64, 256
EPS = 1e-3
C = 128                 # chunk length
NCH = L // C            # 32 chunks
NCORES = 8
PAIRS_PER_CORE = (B * H) // NCORES  # 2
F32 = mybir.dt.float32
BF16 = mybir.dt.bfloat16
VW = 68                 # v_aug row width: 64 v + 1 ones + 3 zero pad


def _legalize_sync_waits(nc):
    """Split multi-wait instructions into preceding single-wait
    EventSemaphore ops on the same engine (same-engine execution is
    in-order, so sequential waits == AND of waits)."""
    for f in nc.m.functions:
        for b in f.blocks:
            insts = b.instructions
            new = []
            dirty = False
            for ins in insts:
                si = ins.sync_info
                if si is not None and si.on_wait is not None and len(si.on_wait) > 1:
                    waits = list(si.on_wait)
                    for j, wt in enumerate(waits[:-1]):
                        es = mybir.InstEventSemaphore(
                            name=f"{ins.name}_xw{j}",
                            engine=ins.engine,
                            ins=[],
                            outs=[],
                            sync_info=mybir.SyncInfo(on_wait=[wt], on_update=[]),
                        )
                        new.append(es)
                    ins.sync_info = mybir.SyncInfo(
                        on_wait=[waits[-1]], on_update=list(si.on_update or [])
                    )
                    dirty = True
                if si is not None and si.on_update is not None and len(si.on_update) > 1:
                    raise AssertionError(
                        f"multi-update on {ins.name} ({ins.opcode}) unsupported"
                    )
                new.append(ins)
            if dirty:
                b.instructions = new


def _build_program(legalize=True):
    nc = bass.Bass()

    xtb_d = nc.dram_tensor("xtb", [128, PAIRS_PER_CORE * L], BF16, kind="ExternalInput")
    vp_d = []
    for p in range(PAIRS_PER_CORE):
        vp_d.append(nc.dram_tensor(f"vp{p}", [128, NCH * VW], BF16, kind="ExternalInput"))
    out_d = nc.dram_tensor(
        "o", [128, NCH * PAIRS_PER_CORE * VW], BF16, kind="ExternalOutput"
    )
    projs_d = nc.dram_tensor("projs", [128, M], BF16, kind="ExternalInput")
    mask_d = nc.dram_tensor("mask2", [C, 2 * C], BF16, kind="ExternalInput")
    epso_d = nc.dram_tensor("epsones", [128, 128], BF16, kind="ExternalInput")

    with tile.TileContext(nc) as tc:
        with (
            tc.tile_pool(name="consts", bufs=1) as consts,
            tc.tile_pool(name="feat", bufs=2) as feat,
            tc.tile_pool(name="kplp", bufs=2) as kplp,
            tc.tile_pool(name="state", bufs=2) as state,
            tc.tile_pool(name="persist", bufs=1) as persist,
            tc.tile_pool(name="psF", bufs=3, space="PSUM") as psF,
            tc.tile_pool(name="psKP", bufs=2, space="PSUM") as psKP,
            tc.tile_pool(name="psKV", bufs=1, space="PSUM") as psKV,
            tc.tile_pool(name="psAP", bufs=2, space="PSUM") as psAP,
        ):
            # ---- resident inputs / constants ----
            # small constants FIRST: the very first feature matmul needs
            # projs, and the HWDGE ring drains in emission order
            projs = consts.tile([128, M], BF16)
            nc.sync.dma_start(projs[:], projs_d[:])
            mask2 = consts.tile([C, 2 * C], BF16)
            nc.sync.dma_start(mask2[:], mask_d[:])
            epso = consts.tile([128, 128], BF16)
            nc.sync.dma_start(epso[:], epso_d[:])
            # xtb: [128, pair, L]; rows 0:64 = rot_q^T, rows 64:128 = rot_k^T
            # sub-DMAs ordered so BOTH pairs' early chunks land first
            # (pair 1's chunk 0 lives at flat col 4096)
            xtb = consts.tile([128, PAIRS_PER_CORE, L], BF16, name="xtb", tag="xtb")
            for g in (0, 4):
                nc.sync.dma_start(
                    xtb[:].rearrange("p a l -> p (a l)")[:, g * 1024 : (g + 1) * 1024],
                    xtb_d[:, g * 1024 : (g + 1) * 1024],
                )
            half = (NCH // 2) * VW
            vp_all = consts.tile([128, PAIRS_PER_CORE, NCH, VW], BF16,
                                 name="vpall", tag="vpall")
            for p in range(PAIRS_PER_CORE):
                nc.sync.dma_start(
                    vp_all[:, p, 0 : NCH // 2, :],
                    vp_d[p][:, 0:half].rearrange("p (c w) -> p c w", w=VW),
                )
            for g in (1, 5, 2, 6, 3, 7):
                nc.sync.dma_start(
                    xtb[:].rearrange("p a l -> p (a l)")[:, g * 1024 : (g + 1) * 1024],
                    xtb_d[:, g * 1024 : (g + 1) * 1024],
                )
            for p in range(PAIRS_PER_CORE):
                nc.sync.dma_start(
                    vp_all[:, p, NCH // 2 : NCH, :],
                    vp_d[p][:, half : 2 * half].rearrange("p (c w) -> p c w", w=VW),
                )
            # resident output accumulation buffer [l, chunk, (pair, 68)]
            obuf = consts.tile([128, NCH, PAIRS_PER_CORE * VW], BF16,
                               name="obuf", tag="obuf")

            # KV state, both pairs in one bank: pair p half h at col (2p+h)*68
            kv_ps = psKV.tile([128, 4 * VW], F32, name="kvps", tag="kvps")
            kv_sb = persist.tile([128, 4 * VW], BF16, name="kvsb", tag="kvsb")

            def stage_b1(ci, fs):
                """In-chunk quadratic AT[lk, lq] for chunk ci (PE only);
                inputs were produced last iteration so these run instantly
                at the head of this iteration's PE queue."""
                t = psAP.tile([128, 392], F32, tag="atpo", name=f"ap{ci}")
                for p in range(PAIRS_PER_CORE):
                    for h in range(2):
                        nc.tensor.matmul(
                            t[:, p * 128 : (p + 1) * 128],
                            fs[:, 1, h * 256 + p * 128 : h * 256 + (p + 1) * 128],
                            fs[:, 0, h * 256 + p * 128 : h * 256 + (p + 1) * 128],
                            start=(h == 0), stop=(h == 1),
                        )
                return t

            def emit_atsb(ci, t):
                """Masked bf16 copy of AT (DVE) — emitted between fs_k and
                fs_q so the DVE stream stays dense."""
                at_sb = state.tile([C, 2, C], BF16, tag="atsb", name=f"at{ci}")
                nc.vector.tensor_tensor(
                    at_sb[:].rearrange("p a l -> p (a l)"),
                    t[:, 0:256], mask2[:], mybir.AluOpType.mult,
                )
                return at_sb

            def stage_b2(ci, fs, kpl, t, at_sb):
                """po output + KV state update for chunk ci, both pairs."""
                for p in range(PAIRS_PER_CORE):
                    po = t[:, 256 + p * VW : 256 + (p + 1) * VW]
                    vslice = vp_all[:, p, ci, :]
                    if ci > 0:
                        for h in range(2):
                            nc.tensor.matmul(
                                po,
                                fs[:, 0, h * 256 + p * 128 : h * 256 + (p + 1) * 128],
                                kv_sb[:, (2 * p + h) * VW : (2 * p + h + 1) * VW],
                                start=(h == 0), stop=False,
                            )
                        nc.tensor.matmul(
                            po, at_sb[:, p, :], vslice, start=False, stop=True
                        )
                    else:
                        nc.tensor.matmul(
                            po, at_sb[:, p, :], vslice, start=True, stop=True
                        )
                # KV += kp^T v_aug + EPS * colsum(v_aug)
                if ci < NCH - 1:
                    for p in range(PAIRS_PER_CORE):
                        vslice = vp_all[:, p, ci, :]
                        for h in range(2):
                            nc.tensor.matmul(
                                kv_ps[:, (2 * p + h) * VW : (2 * p + h + 1) * VW],
                                kpl[:, p, h, :], vslice,
                                start=(ci == 0 and p == 0 and h == 0), stop=True,
                                skip_group_check=True,
                            )
                    # one EPS mm for BOTH pairs: flat [128, 272] out, the
                    # v slice broadcast over the half-dup dim per pair
                    nc.tensor.matmul(
                        kv_ps[:].rearrange("p (a b w) -> p a b w", a=2, b=2),
                        epso[:],
                        vp_all[:, :, ci : ci + 1, :]
                            .broadcast_to([128, PAIRS_PER_CORE, 2, VW]),
                        start=False, stop=True,
                        skip_group_check=True,
                    )
                nc.scalar.activation(
                    obuf[:, ci, :], t[:, 256:392],
                    mybir.ActivationFunctionType.Copy,
                )

            # Software pipeline, one chunk of lookahead. Per-iteration
            # engine streams (emission order == queue order per engine):
            #   ACT: snapshot(ci-1), kpl relu(ci), obuf copy(ci-1)
            #   PE : AT(ci-1), pfK(ci), pfQ(ci), pfkp(ci), po(ci-1), kv(ci-1)
            #   DVE: fs_k(ci), at_sb mult(ci-1), fs_q(ci)
            def iteration(ci, prev):
                lo = ci * C
                if prev is not None:
                    nc.scalar.activation(
                        kv_sb[:], kv_ps[:], mybir.ActivationFunctionType.Copy
                    )
                    t = stage_b1(prev[0], prev[1])
                pfK = psF.tile([128, 512], F32, tag="pfF", name=f"pfK{ci}")
                for h in range(2):
                    nc.tensor.matmul(
                        pfK[:, h * 256 : (h + 1) * 256],
                        projs[D : 2 * D, h * 128 : (h + 1) * 128],
                        xtb[D : 2 * D, :, lo : lo + C],
                        start=True, stop=True,
                    )
                fs = feat.tile([128, 2, 512], BF16, tag="fs", name=f"fs{ci}")
                nc.vector.tensor_scalar(
                    fs[:, 1, :], pfK[:],
                    0.0, EPS, mybir.AluOpType.max, mybir.AluOpType.add,
                )
                if prev is not None:
                    at_sb = emit_atsb(prev[0], t)
                pfQ = psF.tile([128, 512], F32, tag="pfF", name=f"pfQ{ci}")
                for h in range(2):
                    nc.tensor.matmul(
                        pfQ[:, h * 256 : (h + 1) * 256],
                        projs[0:D, h * 128 : (h + 1) * 128],
                        xtb[0:D, :, lo : lo + C],
                        start=True, stop=True,
                    )
                nc.vector.tensor_scalar(
                    fs[:, 0, :], pfQ[:],
                    0.0, EPS, mybir.AluOpType.max, mybir.AluOpType.add,
                )
                if ci < NCH - 1:
                    pfkp = psKP.tile([128, 512], F32, tag="pfkp", name=f"pfkp{ci}")
                    for p in range(PAIRS_PER_CORE):
                        nc.tensor.matmul(
                            pfkp[:, p * 256 : (p + 1) * 256],
                            xtb[D : 2 * D, p, lo : lo + C],
                            projs[D : 2 * D, :],
                            start=True, stop=True,
                        )
                    kpl = kplp.tile([128, 2, 2, C], BF16, tag="kpl", name=f"kpl{ci}")
                    nc.scalar.activation(
                        kpl[:], pfkp[:].rearrange("p (a b m) -> p a b m", a=2, b=2),
                        mybir.ActivationFunctionType.Relu,
                    )
                else:
                    kpl = None
                if prev is not None:
                    stage_b2(prev[0], prev[1], prev[2], t, at_sb)
                return (ci, fs, kpl)

            prev = None
            for ci in range(NCH):
                prev = iteration(ci, prev)
                # drain finished output chunks early; keep the final DMA tiny
                drains = {
                    NCH // 4 + 1: (0, NCH // 4),
                    NCH // 2 + 1: (NCH // 4, NCH // 2),
                    3 * NCH // 4 + 1: (NCH // 2, 3 * NCH // 4),
                    NCH - 3: (3 * NCH // 4, NCH - 4),
                    NCH - 1: (NCH - 4, NCH - 2),
                }
                if ci in drains:
                    qs, qe = drains[ci]
                    nc.sync.dma_start(
                        out_d[:, qs * 2 * VW : qe * 2 * VW],
                        obuf[:, qs:qe, :],
                    )
            # trailing stage_b for the last chunk
            nc.scalar.activation(
                kv_sb[:], kv_ps[:], mybir.ActivationFunctionType.Copy
            )
            t = stage_b1(prev[0], prev[1])
            at_sb = emit_atsb(prev[0], t)
            stage_b2(prev[0], prev[1], prev[2], t, at_sb)
            nc.sync.dma_start(
                out_d[:, (NCH - 2) * 2 * VW :],
                obuf[:, NCH - 2 :, :],
            )

    if legalize:
        _legalize_sync_waits(nc)
    return nc


_PROGRAM_CACHE = {}


def _get_program():
    if "nc" not in _PROGRAM_CACHE:
        _PROGRAM_CACHE["nc"] = _build_program()
    return _PROGRAM_CACHE["nc"]


def _host_rotary(q, k, sinu_pos):
    """Apply GPT-J rotary on host in fp32, return rot_q, rot_k [B,L,H,D]."""
    sinu = np.asarray(sinu_pos, np.float32)[0]          # [L, D]
    half = D // 2
    sin_i = np.repeat(sinu[:, :half], 2, axis=-1)       # [L, D]
    cos_i = np.repeat(sinu[:, half:], 2, axis=-1)

    def rot(t):
        t = np.asarray(t, np.float32)
        r = np.empty_like(t)
        r[..., 0::2] = -t[..., 1::2]
        r[..., 1::2] = t[..., 0::2]
        c = cos_i[None, :, None, :]
        s = sin_i[None, :, None, :]
        return t * c + r * s

    return rot(q), rot(k)


def build_in_maps(q, k, v, sinu_pos, proj):
    bf = ml_dtypes.bfloat16
    rq, rk = _host_rotary(q, k, sinu_pos)
    v = np.asarray(v, np.float32)
    proj = np.asarray(proj, np.float32)

    ratio = 1.0 / np.sqrt(np.float32(M))
    projs = np.zeros((128, M), np.float32)
    projs[0:D, :] = ratio * proj.T
    projs[D : 2 * D, :] = ratio * proj.T
    mask2 = np.tile(np.triu(np.ones((C, C), np.float32)), (1, 2))
    epsones = np.full((128, 128), EPS, np.float32)

    pairs = [(b, h) for b in range(B) for h in range(H)]
    in_maps = []
    for core in range(NCORES):
        im = {
            "projs": projs.astype(bf),
            "mask2": mask2.astype(bf),
            "epsones": epsones.astype(bf),
        }
        xtb = np.empty((128, PAIRS_PER_CORE, L), np.float32)
        for p in range(PAIRS_PER_CORE):
            b, h = pairs[core * PAIRS_PER_CORE + p]
            xtb[0:D, p, :] = rq[b, :, h, :].T
            xtb[D : 2 * D, p, :] = rk[b, :, h, :].T
            vz = np.zeros((C, NCH, VW), np.float32)
            vz[:, :, 0:D] = v[b, :, h, :].reshape(NCH, C, D).transpose(1, 0, 2)
            vz[:, :, D] = 1.0
            im[f"vp{p}"] = np.ascontiguousarray(
                vz.reshape(C, NCH * VW)
            ).astype(bf)
        im["xtb"] = np.ascontiguousarray(
            xtb.reshape(128, PAIRS_PER_CORE * L)
        ).astype(bf)
        in_maps.append(im)
    return in_maps


def kernel(q, k, v, sinu_pos, proj):
    nc = _get_program()
    in_maps = build_in_maps(q, k, v, sinu_pos, proj)
    res = run_bass_kernel_spmd(nc, in_maps, core_ids=list(range(NCORES)))

    pairs = [(b, h) for b in range(B) for h in range(H)]
    out = np.empty((B, L, H, D), np.float32)
    for core in range(NCORES):
        ob = np.asarray(res.results[core]["o"], dtype=np.float32).reshape(
            128, NCH, PAIRS_PER_CORE, VW
        )
        for p in range(PAIRS_PER_CORE):
            b, h = pairs[core * PAIRS_PER_CORE + p]
            x = ob[:, :, p, :].transpose(1, 0, 2).reshape(L, VW)  # [L, 68]
            out[b, :, h, :] = x[:, 0:D] / x[:, D : D + 1]
    return out
